# revision 1
# baseline (speedup 1.0000x reference)
"""Trainium2 Bass kernel for nn_ArithmeticNps (moe_routing) — v3.

Strategy
--------
Pure data parallel over 8 NeuronCores; per-core batch 2048 in 4 chunks of
512 (PSUM-bank-width columns). All encoder/selector algebra is folded on
the host (fp64, weights-only):

* All MLP biases are zero in this problem, so x1e(op1) is exactly
  piecewise-linear with ONE breakpoint: x1e = a_pos*relu(op1) +
  a_neg*min(op1,0). x2e(op2) = w2e^T relu(op2*w0 + w1) needs only the
  64-dim relu r2. ope(opr) is one of 3 fixed vectors -> selector-1 scores
  become three tiny folded matmuls (slot0 from [op1+,op1-,1], slot1 from
  r2, slot2 via an exact quadratic in opr), all fp32 (routing needs ~1e-7
  accuracy; min top-2 gap is 6e-7).
* A provable superset A of rules that can win the flat argmax is computed
  from the weights alone (upper envelopes per slot, eps-tolerant exact
  line-envelope math). Here |A| = 6 of 16 -> scores and the heavy per-rule
  FFN run over A only, masked exactly by the ReLU-penalty trick.
* The var_p/var_c slot selection is folded INTO the FFN first layer:
  in_p = [op1*snp; op1p*snp; r2*sp; ohK-1], with host-folded lhsT
  [a_neg@W1a; (a_pos-a_neg)@W1a; w2e@W1a; PEN-rows]. rule_W2 is folded
  with dec_w1 so the FFN second layer accumulates straight into the
  64-dim decoder hidden. Value path runs in float32r (1 cyc/row).
* Chunks are software-pipelined three deep (A: input matmuls, B1:
  selector chain, C: FFN+decoder, B2: FFN-input build) emitted as
  A(i) B1(i-1) C(i-2) B2(i-1) so the PE never waits on a chunk's own
  vector chain. Selector-2 sums go through Pool partition_all_reduce
  into a 32-aligned layout so one is_gt/is_le yields both select rows
  at matmul-legal base partitions.
"""

import os
import sys

sys.path.insert(0, "/opt/trn_rl_repo")

import numpy as np

REPEAT = int(os.environ.get("NPS_REPEAT", "1"))

NCORES = 8
B_FULL = 16384
BC = B_FULL // NCORES  # per-core batch
CHUNK = 512
NCHUNK = BC // CHUNK
NR = 16
CV = 128
CM = 128
PEN = 32768.0


# ---------------------------------------------------------------------------
# host algebra
# ---------------------------------------------------------------------------

def _tables(p):
    f8 = np.float64
    w0 = p["enc_op_w1"][0].astype(f8)
    w1 = p["enc_op_w1"][1].astype(f8)
    b1e = p["enc_op_b1"].astype(f8)
    w2e = p["enc_op_w2"].astype(f8)
    b2e = p["enc_op_b2"].astype(f8)
    w1o = p["enc_opr_w1"].astype(f8)
    b1o = p["enc_opr_b1"].astype(f8)
    w2o = p["enc_opr_w2"].astype(f8)
    b2o = p["enc_opr_b2"].astype(f8)
    assert np.all(b1e == 0.0), "nonzero enc_op_b1 breaks the x1e fold"

    a_pos = np.maximum(w0, 0.0) @ w2e
    a_neg = np.minimum(w0, 0.0) @ w2e
    c1 = b2e
    ope = np.maximum(w1o + b1o[None, :], 0.0) @ w2o + b2o  # (3,128)

    read1 = (np.einsum("nr,nrm->nm", p["rules_emb"].astype(f8),
                       p["s1_k_w"].astype(f8)) + p["s1_k_b"].astype(f8))
    G = read1 @ p["s1_q_w"].astype(f8).T            # (16,128)
    att1b = read1 @ p["s1_q_b"].astype(f8)          # (16,)
    u_pos, u_neg, u_c1 = G @ a_pos, G @ a_neg, G @ c1
    C2 = G @ w2e.T                                  # (16,64)
    c2c = G @ b2e
    V = ope @ G.T                                   # (3,16)

    r2t = (np.einsum("rc,ncm->rnm", p["rules_emb"].astype(f8),
                     p["s2_k_w"].astype(f8)) + p["s2_k_b"].astype(f8))
    s2q = p["s2_q_w"].astype(f8)
    s2qb = p["s2_q_b"].astype(f8)
    q0, q1 = s2q[0].T, s2q[1].T
    P_pos = np.einsum("rnm,m->rn", r2t, q0 @ a_pos)  # (16,2)
    P_neg = np.einsum("rnm,m->rn", r2t, q0 @ a_neg)
    P_c = np.einsum("rnm,m->rn", r2t, q0 @ c1)
    B2 = np.einsum("rnm,mj->rnj", r2t, q1 @ w2e.T)   # (16,2,64)
    B2c = np.einsum("rnm,m->rn", r2t, q1 @ b2e)
    att2b = np.einsum("rnm,km->rnk", r2t, s2qb)      # (16,2,2)

    return dict(w0=w0, w1=w1, a_pos=a_pos, a_neg=a_neg, w2e=w2e, ope=ope,
                u_pos=u_pos, u_neg=u_neg, u_c1=u_c1, C2=C2, c2c=c2c, V=V,
                att1b=att1b, P_pos=P_pos, P_neg=P_neg, P_c=P_c, B2=B2,
                B2c=B2c, att2b=att2b)


def _active_rules(t, eps=1e-4):
    """Superset of rules that can win the flat argmax (weights only)."""
    A = set()
    if np.any(t["u_c1"] != 0.0):
        return list(range(NR))
    up, un = t["u_pos"], t["u_neg"]
    A |= set(np.nonzero(up >= up.max() - eps)[0].tolist())
    A |= set(np.nonzero(un <= un.min() + eps)[0].tolist())
    A.add(0)  # op1 == 0 tie goes to first index within slot 0
    for o in range(3):
        v = t["V"][o]
        A |= set(np.nonzero(v >= v.max() - eps)[0].tolist())
    w0, w1, C2, c2c = t["w0"], t["w1"], t["C2"], t["c2c"]
    bps = sorted({(-w1[j] / w0[j]) for j in range(64) if w0[j] != 0.0})
    LIM = 1e6
    edges = [-LIM] + [b for b in bps if -LIM < b < LIM] + [LIM]
    for a, b in zip(edges[:-1], edges[1:]):
        if b - a < 1e-12:
            continue
        mid = 0.5 * (a + b)
        act = (mid * w0 + w1) > 0.0
        sl = C2[:, act] @ w0[act]
        ic = C2[:, act] @ w1[act] + c2c
        xs = [a, b]
        for i in range(NR):
            for j in range(i + 1, NR):
                ds = sl[i] - sl[j]
                if ds != 0.0:
                    x = (ic[j] - ic[i]) / ds
                    if a < x < b:
                        xs.append(x)
        xs = np.array(xs)
        sc = sl[:, None] * xs[None, :] + ic[:, None]
        mx = sc.max(axis=0)
        A |= set(np.nonzero(np.any(sc >= mx[None, :] - eps, axis=1))[0].tolist())
    return sorted(A)


def _host_prep(p):
    f4 = np.float32
    f8 = np.float64
    t = _tables(p)
    A = _active_rules(t)
    K = len(A)
    assert 3 * K <= 32
    # Engine partition-access rule: base must be 0/32/64/96 (<=32 rows),
    # 0/64 (<=64), 0 (>64).  RT PSUM rows: scores (slot-major s*K+ai) at
    # 0:3K | att2 p-pairs 32:32+2K | att2 c-pairs 64:64+2K | zero pad to
    # 96.  M1/M2 cover rows 0:96 in one accumulation group.
    RT1 = 96

    use_rb1 = bool(np.any(p["rule_b1"]))
    use_rb2 = bool(np.any(p["rule_b2"]))
    use_d1b = bool(np.any(p["dec_b1"]))
    decb2 = float(np.asarray(p["dec_b2"]).reshape(-1)[0])
    # in_p rows: 0:64 r2*sp | 64:64+K ohK-1 | pad zeros | 96:98 op1 rows
    PH = 98

    # ---- M1 (5 x RT1): rhs rows [op1p; op1n; ones; opr; opr^2] ----------
    # slot2 scores are the exact quadratic through V[0..2] on the
    # ones/opr/opr^2 rows, so they fold into the same matmul.  att1b (the
    # folded s1_q_b term) is a per-rule constant added to all 3 slots.
    M1 = np.zeros((5, RT1), f8)
    M1[0, 0:K] = t["u_pos"][A]
    M1[1, 0:K] = t["u_neg"][A]
    M1[2, 0:K] = t["u_c1"][A] + t["att1b"][A]
    M1[2, K:2 * K] = t["c2c"][A] + t["att1b"][A]
    V = t["V"][:, A]
    M1[2, 2 * K:3 * K] = V[0] + t["att1b"][A]
    M1[3, 2 * K:3 * K] = (4.0 * V[1] - 3.0 * V[0] - V[2]) / 2.0
    M1[4, 2 * K:3 * K] = (V[2] + V[0] - 2.0 * V[1]) / 2.0
    for ai, r in enumerate(A):
        for n in range(2):
            jp = 32 + 2 * ai + (0 if n == 0 else 32)
            M1[0, jp + 0] = t["P_pos"][r, n]
            M1[1, jp + 0] = t["P_neg"][r, n]
            M1[2, jp + 0] = t["P_c"][r, n] + t["att2b"][r, n, 0]
            M1[2, jp + 1] = t["B2c"][r, n] + t["att2b"][r, n, 1]

    # ---- M2 (64 x RT1): rhs r2 ------------------------------------------
    M2 = np.zeros((64, RT1), f8)
    M2[:, K:2 * K] = t["C2"][A].T
    for ai, r in enumerate(A):
        for n in range(2):
            jp = 32 + 2 * ai + (0 if n == 0 else 32)
            M2[:, jp + 1] = t["B2"][r, n]

    # ---- rep2 (3K x 96): eq rows -> [signed att2 mask | raw one-hot,
    # zero-padded to 32 rows so oh32/ohm1 cover in_p rows 64:96] ----------
    # Ties across slots of the same rule only scale the (sign-decided)
    # att2 sums; the raw one-hot is clamped by is_ge(.,0.5) afterwards.
    rep2 = np.zeros((3 * K, 96), f8)
    for s in range(3):
        for ai in range(K):
            row = s * K + ai
            rep2[row, 2 * ai + 0] = -1.0
            rep2[row, 2 * ai + 1] = 1.0
            rep2[row, 32 + 2 * ai + 0] = -1.0
            rep2[row, 32 + 2 * ai + 1] = 1.0
            rep2[row, 64 + ai] = 1.0

    # ---- fold (64 x 66): sums of the signed-masked att2 rows ------------
    # dps cols: 0,1 = [dp; dc] (sel), 32,33 = [dp; dp] (snp), 64,65 =
    # [dc; dc] (snc) -- all read/write bases land on 0/32/64.
    fold = np.zeros((64, 66), f8)
    fold[0:32, 0] = 1.0
    fold[32:64, 1] = 1.0
    fold[0:32, 32] = 1.0
    fold[0:32, 33] = 1.0
    fold[32:64, 64] = 1.0
    fold[32:64, 65] = 1.0

    # ---- FFN folded weights (PH=98 layout) ------------------------------
    # rule_b1 folds into the pen rows via alpha*(sum(oh)-K) since
    # sum(oh)==1 after the clamp.
    W1P = np.zeros((PH, 128 * K), f8)
    W1C = np.zeros((PH, 128 * K), f8)
    W2D = np.zeros((128, 64 * K), f8)
    dec_w1 = p["dec_w1"].astype(f8)
    for ai, r in enumerate(A):
        W1a = p["rule_W1"][r][:128].astype(f8)
        W1b = p["rule_W1"][r][128:].astype(f8)
        cs = slice(128 * ai, 128 * ai + 128)
        W1P[0:64, cs] = t["w2e"] @ W1a
        W1P[64 + ai, cs] += PEN
        if use_rb1:
            W1P[64:64 + K, cs] += (p["rule_b1"][r].astype(f8)[None, :]
                                   / (1.0 - K))
        W1P[96, cs] = t["a_neg"] @ W1a
        W1P[97, cs] = (t["a_pos"] - t["a_neg"]) @ W1a
        W1C[0:64, cs] = t["w2e"] @ W1b
        W1C[96, cs] = t["a_neg"] @ W1b
        W1C[97, cs] = (t["a_pos"] - t["a_neg"]) @ W1b
        W2D[:, 64 * ai:64 * ai + 64] = p["rule_W2"][r].astype(f8) @ dec_w1
    rb2d = (p["rule_b2"][A].astype(f8) @ dec_w1)  # (K,64)

    # ---- pack fp32 consts: cf (128, NCF) --------------------------------
    off = {}
    ncol = 0

    def alloc(name, cols):
        nonlocal ncol
        off[name] = ncol
        ncol += cols

    alloc("w0T", 64)      # row 32 (lhsT base matches rhs X[32:33])
    alloc("w1col", 1)     # rows 0:64
    alloc("M1", RT1)      # rows 0:5
    alloc("M2", RT1)      # rows 0:64
    alloc("fold", 66)     # rows 0:64
    alloc("negone", 1)    # rows 0:32
    if use_d1b:
        alloc("dec1b", 1)
    cf = np.zeros((128, ncol), f4)
    cf[32:33, off["w0T"]:off["w0T"] + 64] = t["w0"][None, :]
    cf[0:64, off["w1col"]] = t["w1"]
    cf[0:5, off["M1"]:off["M1"] + RT1] = M1
    cf[0:64, off["M2"]:off["M2"] + RT1] = M2
    cf[0:64, off["fold"]:off["fold"] + 66] = fold
    cf[0:32, off["negone"]] = -1.0
    if use_d1b:
        cf[0:64, off["dec1b"]] = p["dec_b1"].astype(f8)

    # ---- pack f32r consts: cr (128, NCR); small head first so the FFN
    # weight DMAs can trail the first chunks ------------------------------
    roff = {}
    rcol = 0

    def ralloc(name, cols):
        nonlocal rcol
        roff[name] = rcol
        rcol += cols

    ralloc("rep2", 96)
    ralloc("ones2", 128)   # rows 32:34: [sp->cols 0:64; sc->cols 64:128]
    ralloc("dec2", 1)
    if use_rb2:
        ralloc("rb2d", 64)
    head = rcol
    ralloc("W1P", 128 * K)
    ralloc("W1C", 128 * K)
    ralloc("W2D", 64 * K)
    cr = np.zeros((128, rcol), f4)
    cr[0:3 * K, roff["rep2"]:roff["rep2"] + 96] = rep2
    cr[32, roff["ones2"]:roff["ones2"] + 64] = 1.0
    cr[33, roff["ones2"] + 64:roff["ones2"] + 128] = 1.0
    cr[0:64, roff["dec2"]] = p["dec_w2"].astype(f8)[:, 0]
    if use_rb2:
        cr[0:K, roff["rb2d"]:roff["rb2d"] + 64] = rb2d
    cr[0:PH, roff["W1P"]:roff["W1P"] + 128 * K] = W1P
    cr[0:PH, roff["W1C"]:roff["W1C"] + 128 * K] = W1C
    cr[0:128, roff["W2D"]:roff["W2D"] + 64 * K] = W2D

    # ---- per-example input rows (host, element-wise only) ---------------
    op1 = np.asarray(p["operand1"], f4)
    op2 = np.asarray(p["operand2"], f4)
    opr = np.asarray(p["operator"]).astype(f4)
    xin = np.zeros((NCORES, 8, BC), f4)
    xin[:, 0] = np.maximum(op1, 0.0).reshape(NCORES, BC)
    xin[:, 1] = np.minimum(op1, 0.0).reshape(NCORES, BC)
    xin[:, 2] = 1.0
    xin[:, 3] = opr.reshape(NCORES, BC)
    xin[:, 4] = (opr * opr).reshape(NCORES, BC)
    xin[:, 5] = op2.reshape(NCORES, BC)
    xin[:, 6] = op1.reshape(NCORES, BC)
    xin[:, 7] = np.maximum(op1, 0.0).reshape(NCORES, BC)

    return dict(cf=np.ascontiguousarray(cf), cr=np.ascontiguousarray(cr),
                xin=xin, off=off, roff=roff, rhead=head, K=K, A=A, RT1=RT1,
                PH=PH, use_rb1=use_rb1, use_rb2=use_rb2,
                use_d1b=use_d1b, decb2=decb2)


# ---------------------------------------------------------------------------
# device kernel
# ---------------------------------------------------------------------------

def _build(consts):
    import concourse.bacc as bacc
    import concourse.tile as tile
    from concourse import bass_isa, mybir

    f32 = mybir.dt.float32
    f32r = mybir.dt.float32r
    AF = mybir.ActivationFunctionType
    ALU = mybir.AluOpType

    K = consts["K"]
    RT1 = consts["RT1"]
    PH = consts["PH"]
    off = consts["off"]
    roff = consts["roff"]
    C = CHUNK

    nc = bacc.Bacc("TRN2", target_bir_lowering=False, debug=False)

    xin_d = nc.dram_tensor("xin", [8, BC], f32, kind="ExternalInput").ap()
    cf_d = nc.dram_tensor("cf", list(consts["cf"].shape), f32,
                          kind="ExternalInput").ap()
    cr_d = nc.dram_tensor("cr", list(consts["cr"].shape), f32r,
                          kind="ExternalInput").ap()
    out_d = nc.dram_tensor("out", [1, BC], f32, kind="ExternalOutput").ap()

    with tile.TileContext(nc) as tc:
        with tc.tile_pool(name="wsb", bufs=1) as wsb, \
             tc.tile_pool(name="xsb", bufs=2) as xsb, \
             tc.tile_pool(name="asb", bufs=2) as asb, \
             tc.tile_pool(name="vsb", bufs=2) as vsb, \
             tc.tile_pool(name="msb", bufs=3) as msb, \
             tc.tile_pool(name="osb", bufs=2) as osb, \
             tc.tile_pool(name="prt", bufs=2, space="PSUM") as prt, \
             tc.tile_pool(name="po", bufs=2, space="PSUM") as po, \
             tc.tile_pool(name="ppre", bufs=3, space="PSUM") as ppre, \
             tc.tile_pool(name="pacc", bufs=1, space="PSUM") as pacc:

            CF = wsb.tile(list(consts["cf"].shape), f32, tag="cf")
            nc.sync.dma_start(CF[:], cf_d[:])
            CR = wsb.tile(list(consts["cr"].shape), f32r, tag="cr")
            ncr = consts["cr"].shape[1]
            head = consts["rhead"]
            nc.sync.dma_start(CR[:, 0:head], cr_d[:, 0:head])
            half = head + (ncr - head) // 2
            nc.gpsimd.dma_start(CR[:, head:half], cr_d[:, head:half])
            nc.gpsimd.dma_start(CR[:, half:ncr], cr_d[:, half:ncr])

            # PE pstate warmup: junk matmuls with no DMA dependency so the
            # tensor engine ramps to full clock while weights stream in.
            wua = wsb.tile([1, 128], f32, tag="wua")
            nc.vector.memset(wua[:], 1.0)
            wub = wsb.tile([1, 128], f32, tag="wub")
            nc.vector.memset(wub[:], 0.0)
            wups = ppre.tile([128, C], f32, tag="pre")
            for _ in range(10):
                nc.tensor.matmul(wups[:, 0:128], wua[:], wub[:], start=True,
                                 stop=True)

            def cfs(name, p0, p1, c0=0, c1=None):
                c = off[name]
                w = {"w0T": 64, "M1": RT1, "M2": RT1, "fold": 66,
                     "w1col": 1, "negone": 1, "dec1b": 1}[name]
                if c1 is None:
                    c1 = w
                return CF[p0:p1, c + c0:c + c1]

            def stage_a(cs):
                """Input DMA + all input-side matmuls."""
                X = xsb.tile([33, C], f32, tag="X")
                nc.sync.dma_start(X[0:5, :], xin_d[0:5, cs])
                nc.sync.dma_start(X[32:33, :], xin_d[5:6, cs])
                Xop = xsb.tile([2, C], f32, tag="Xop")
                nc.sync.dma_start(Xop[:], xin_d[6:8, cs])

                p2 = po.tile([64, C], f32, tag="po")
                nc.tensor.matmul(p2[:], cfs("w0T", 32, 33), X[32:33, :],
                                 start=True, stop=True)
                r2 = asb.tile([64, C], f32, tag="r2")
                nc.scalar.activation(r2[:], p2[:], AF.Relu,
                                     bias=cfs("w1col", 0, 64))

                RT = prt.tile([RT1, C], f32, tag="rt")
                nc.tensor.matmul(RT[0:RT1, :], cfs("M1", 0, 5), X[0:5, :],
                                 start=True, stop=False)
                nc.tensor.matmul(RT[0:RT1, :], cfs("M2", 0, 64), r2[:],
                                 start=False, stop=True)
                return dict(X=X, Xop=Xop, r2=r2, RT=RT)

            def stage_b1(t):
                """Global-max argmax + selector-2 sums."""
                RT = t["RT"]
                K3 = 3 * K
                sc3 = asb.tile([K3, C], f32, tag="sc3")
                nc.scalar.copy(sc3[:], RT[0:K3, :])
                mx = asb.tile([K3, C], f32, tag="mx")
                nc.gpsimd.partition_all_reduce(mx[:], sc3[:], channels=K3,
                                               reduce_op=bass_isa.ReduceOp.max)
                eq = asb.tile([K3, C], f32r, tag="eq")
                nc.vector.tensor_tensor(eq[:], RT[0:K3, :], mx[:],
                                        op=ALU.is_equal)
                orp = po.tile([96, C], f32, tag="po")
                o1 = roff["rep2"]
                nc.tensor.matmul(orp[:], CR[0:K3, o1:o1 + 96], eq[:],
                                 start=True, stop=True)
                oh = asb.tile([32, C], f32r, tag="oh")
                nc.vector.tensor_scalar(oh[:], orp[64:96, :], 0.5, None,
                                        op0=ALU.is_ge)
                orsb = asb.tile([64, C], f32, tag="orsb")
                nc.scalar.copy(orsb[:], orp[0:64, :])
                mks = asb.tile([64, C], f32, tag="mks")
                nc.vector.tensor_tensor(mks[0:32, :], RT[32:64, :],
                                        orsb[0:32, :], op=ALU.mult)
                nc.vector.tensor_tensor(mks[32:64, :], RT[64:96, :],
                                        orsb[32:64, :], op=ALU.mult)
                dps = po.tile([66, C], f32, tag="po")
                nc.tensor.matmul(dps[:], cfs("fold", 0, 64), mks[:],
                                 start=True, stop=True)
                sel = asb.tile([34, C], f32r, tag="sel")
                nc.vector.tensor_scalar(sel[32:34, :], dps[0:2, :], 0.0,
                                        None, op0=ALU.is_gt)
                snp2 = asb.tile([2, C], f32, tag="snp2")
                nc.vector.tensor_scalar(snp2[:], dps[32:34, :], 0.0, None,
                                        op0=ALU.is_le)
                snc2 = asb.tile([2, C], f32, tag="snc2")
                nc.vector.tensor_scalar(snc2[:], dps[64:66, :], 0.0,
                                        None, op0=ALU.is_le)
                return dict(oh=oh, sel=sel, snp2=snp2, snc2=snc2)

            def stage_b2(ta, tb, idx):
                """FFN input tiles in_p / in_c (persistent, pad pre-zeroed).
                rows 0:64 r2*mask | 64:64+K ohK-1 | 96:98 op1 rows."""
                Xop, r2 = ta["Xop"], ta["r2"]
                oh, sel = tb["oh"], tb["sel"]
                snp2, snc2 = tb["snp2"], tb["snc2"]
                spsc = po.tile([128, C], f32, tag="po")
                o1 = roff["ones2"]
                nc.tensor.matmul(spsc[:], CR[32:34, o1:o1 + 128],
                                 sel[32:34, :], start=True, stop=True)

                inp = vsb.tile([PH, C], f32r, tag="inp")
                inc = vsb.tile([PH, C], f32r, tag="inc")
                nc.gpsimd.tensor_tensor(inp[96:98, :], Xop[:], snp2[:],
                                        op=ALU.mult)
                nc.gpsimd.tensor_tensor(inc[96:98, :], Xop[:], snc2[:],
                                        op=ALU.mult)
                nc.vector.tensor_tensor(inp[0:64, :], r2[:], spsc[0:64, :],
                                        op=ALU.mult)
                nc.vector.tensor_tensor(inc[0:64, :], r2[:], spsc[64:128, :],
                                        op=ALU.mult)
                nc.scalar.activation(inp[64:96, :], oh[:], AF.Identity,
                                     bias=cfs("negone", 0, 32))
                nc.scalar.activation(inc[64:96, :], oh[:], AF.Identity,
                                     bias=cfs("negone", 0, 32))
                return dict(inp=inp, inc=inc, oh=oh)

            def stage_c(t, cs, parity=0, mid=None):
                """Rule FFN over the active set + decoder. ``mid`` emits the
                next chunk's B2 stage between rules so its spsc matmul never
                exposes a PE stall."""
                inp, inc, oh = t["inp"], t["inc"], t["oh"]
                mid_out = None
                d1 = pacc.tile([64, C], f32, tag="acc")
                for ai in range(K):
                    pre = ppre.tile([128, C], f32, tag="pre")
                    o1 = roff["W1P"] + 128 * ai
                    o2 = roff["W1C"] + 128 * ai
                    nc.tensor.matmul(pre[:], CR[0:PH, o1:o1 + 128],
                                     inp[0:PH, :], start=True, stop=False)
                    nc.tensor.matmul(pre[:], CR[0:PH, o2:o2 + 128],
                                     inc[0:PH, :], start=False, stop=True)
                    hm = msb.tile([128, C], f32r, tag="hm")
                    if ai == K - 2:
                        nc.vector.tensor_scalar(hm[:], pre[:], 0.0, None,
                                                op0=ALU.max)
                    else:
                        nc.scalar.activation(hm[:], pre[:], AF.Relu)
                    o3 = roff["W2D"] + 64 * ai
                    nc.tensor.matmul(d1[:], CR[0:128, o3:o3 + 64], hm[:],
                                     start=(ai == 0),
                                     stop=(ai == K - 1 and not consts["use_rb2"]))
                    if ai == min(2, K - 1) and mid is not None:
                        mid_out = mid()
                if consts["use_rb2"]:
                    nc.tensor.matmul(d1[:], CR[0:K, roff["rb2d"]:roff["rb2d"] + 64],
                                     oh[:], start=False, stop=True)

                d1sb = vsb.tile([64, C], f32r, tag="d1sb")
                if consts["use_d1b"]:
                    nc.scalar.activation(d1sb[:], d1[:], AF.Relu,
                                         bias=cfs("dec1b", 0, 64))
                else:
                    nc.scalar.activation(d1sb[:], d1[:], AF.Relu)
                x3 = pacc.tile([1, C], f32, tag="acc")
                nc.tensor.matmul(x3[:], CR[0:64, roff["dec2"]:roff["dec2"] + 1],
                                 d1sb[:], start=True, stop=True)
                x3sb = osb.tile([1, C], f32, tag="x3")
                if consts["decb2"] != 0.0:
                    nc.scalar.activation(x3sb[:], x3[:], AF.Identity,
                                         bias=consts["decb2"])
                else:
                    nc.scalar.copy(x3sb[:], x3[:])
                nc.sync.dma_start(out_d[:, cs], x3sb[:])
                return mid_out

            chunks = [slice(ci * C, (ci + 1) * C)
                      for _ in range(REPEAT) for ci in range(NCHUNK)]
            n = len(chunks)
            ta, tb1, tb2 = {}, {}, {}
            for i in range(n):
                ta[i] = stage_a(chunks[i])
                if i >= 1:
                    tb1[i - 1] = stage_b1(ta[i - 1])
                if i >= 2:
                    tb2[i - 1] = stage_c(
                        tb2[i - 2], chunks[i - 2],
                        mid=lambda i=i: stage_b2(ta[i - 1], tb1[i - 1],
                                                 i - 1))
                elif i >= 1:
                    tb2[i - 1] = stage_b2(ta[i - 1], tb1[i - 1], i - 1)
            tb1[n - 1] = stage_b1(ta[n - 1])
            tb2[n - 1] = stage_c(
                tb2[n - 2], chunks[n - 2],
                mid=lambda: stage_b2(ta[n - 1], tb1[n - 1], n - 1))
            stage_c(tb2[n - 1], chunks[n - 1])

    nc.compile()
    return nc


def _make_in_maps(consts, p=None):
    base = {"cf": consts["cf"], "cr": consts["cr"]}
    in_maps = []
    for cidx in range(NCORES):
        m = dict(base)
        m["xin"] = np.ascontiguousarray(consts["xin"][cidx])
        in_maps.append(m)
    return in_maps


def kernel(**inputs):
    from concourse.bass_utils import run_bass_kernel_spmd

    p = {k: np.asarray(v) for k, v in inputs.items()}
    consts = _host_prep(p)
    nc = _build(consts)
    in_maps = _make_in_maps(consts)

    res = run_bass_kernel_spmd(nc, in_maps, core_ids=list(range(NCORES)))
    out = np.concatenate([res.results[i]["out"].reshape(-1)
                          for i in range(NCORES)])
    return out.astype(np.float32)


if __name__ == "__main__":
    sys.path.insert(0, "/root/problem")
    import reference as R

    inp = {k: np.asarray(v) for k, v in R.setup_inputs().items()}
    got = kernel(**inp)
    print("kernel output:", got.shape, got.dtype, got[:5])

